# revision 45
# baseline (speedup 1.0000x reference)
"""Causal multi-head attention (nn_Attention_87840671138123) on 8 trn2 NeuronCores.

Problem (B=2, S=2048, D=1024, H=16 heads, E=64 head_dim), fp32:
    Q = einsum('bsd,hde->bhse', q, W_q)   (same for K, V)
    scores = Q @ K^T / sqrt(D), causal mask, softmax
    attn = probs @ V  -> [B, S, D] (head-major concat)
    out = attn @ W_o.T

Sharding: core = 4*b + quad. Each core handles batch b and a quad of 4 heads
(heads 4*quad .. 4*quad+3). It computes a partial output
    out_part = attn_quad @ W_o.T[quad rows, :]   [S, D]  (fp16)
and the host sums the 4 partials per batch (the "all-reduce" of the output
projection done host-side at gather time).

Device layout choices (per core):
 - Host pre-tiles x^T as [P, NJ, ND, SJ] so each j-tile DMA is one contiguous
   8KB segment per partition (big descriptors); all x tiles are pre-issued at
   kernel start so input DMA never gates compute.
 - QK path in bf16 (fp8 DoubleRow was measured at 1 cyc/col on this HW — no
   win — so bf16 at 1 cyc/col with full-width zero-padded KTH is optimal).
 - Projections produce QT in [head-pair x 64, S] layout and KTH per head
   zero-padded (1e-20) to 128 partitions so the scores matmul runs K=128.
 - V_aug blocks [64 V | 1.0 x 64]: the 1.0 columns accumulate the softmax
   denominator into PSUM rows 64..127 during the attn matmul.
 - Softmax epilogue (den copy -> DMA to partition 0 -> reciprocal ->
   gpsimd partition_broadcast -> multiply) is emitted AFTER the previous
   tile's output projection, so the slow (3.3us) InstReciprocal never sits
   between the PE and the PSUM-bank-freeing casts in the DVE queue.
   partition_broadcast's input must be at physical partition 0 (GPSIMD DSPs
   only reach their own 16 partitions) — hence the DMA bounce.
 - Causal mask multiply for diagonal blocks runs on GPSIMD (idle engine).
 - Output projection one s-tile behind; fp16 partials DMA'd out.
"""

import ml_dtypes
import numpy as np

import concourse.bass as bass
import concourse.tile as tile
from concourse import bacc, mybir
from concourse.bass_utils import run_bass_kernel_spmd

B, S, D, H, E = 2, 2048, 1024, 16, 64
P = 128
NCORES = 8
SJ = 512            # s-tile width
NJ = S // SJ        # 4 s-tiles
ND = D // P         # 8 d-chunks
NT = S // P         # 16 t-chunks
f32 = mybir.dt.float32
bf16 = mybir.dt.bfloat16
fp16 = mybir.dt.float16
EXP = mybir.ActivationFunctionType.Exp
MULT = mybir.AluOpType.mult

QK_DT = bf16
V_DT = fp16

_NP_OF = {bf16: ml_dtypes.bfloat16, fp16: np.float16, f32: np.float32}

_NC_CACHE = []


def _patch_ldw_opt():
    """Enable walrus LDWEIGHTS optimization (fast weight load)."""
    from concourse import bass_utils as _bu
    if getattr(_bu, "_ldw_patched", False):
        return
    _orig_run = _bu.run_command

    def _run(argv, **kw):
        argv = ["--enable-ldw-opt=true" if a == "--enable-ldw-opt=false" else a
                for a in argv]
        return _orig_run(argv, **kw)

    _bu.run_command = _run
    _bu._ldw_patched = True


def _build():
    # NOTE: walrus --enable-ldw-opt rejects this kernel's explicit
    # InstLdweights stream, and traces show the tile-framework Ldweights
    # already overlap fully with MATMULs without it — so no patch here.
    nc = bacc.Bacc("TRN2", target_bir_lowering=False, debug=False)

    qT_d = nc.dram_tensor("qT", [P, NJ, ND, SJ], QK_DT, kind="ExternalInput")
    kT_d = nc.dram_tensor("kT", [P, NJ, ND, SJ], QK_DT, kind="ExternalInput")
    vT_d = nc.dram_tensor("vT", [P, NJ, ND, SJ], V_DT, kind="ExternalInput")
    wq_d = nc.dram_tensor("wq", [P, ND, 4 * E], QK_DT, kind="ExternalInput")
    wk_d = nc.dram_tensor("wk", [P, ND, 4 * E], QK_DT, kind="ExternalInput")
    wv_d = nc.dram_tensor("wv", [P, ND, 4 * E], V_DT, kind="ExternalInput")
    wot_d = nc.dram_tensor("wot", [P, 2, D], V_DT, kind="ExternalInput")
    id_d = nc.dram_tensor("idm", [P, P], QK_DT, kind="ExternalInput")
    npat_d = nc.dram_tensor("npat", [P, P], QK_DT, kind="ExternalInput")
    out_d = nc.dram_tensor("out", [S, D], fp16, kind="ExternalOutput")

    with tile.TileContext(nc) as tc:
        with (
            tc.tile_pool(name="pers", bufs=1) as pers,
            tc.tile_pool(name="xt", bufs=NJ) as xt_pool,
            tc.tile_pool(name="ex", bufs=4) as ex_pool,
            tc.tile_pool(name="sm", bufs=2) as sm_pool,
            tc.tile_pool(name="epi", bufs=1) as epi_pool,
            tc.tile_pool(name="ot", bufs=2) as ot_pool,
            tc.tile_pool(name="pj", bufs=2, space="PSUM") as pj_pool,
            tc.tile_pool(name="sc", bufs=2, space="PSUM") as sc_pool,
            tc.tile_pool(name="at", bufs=4, space="PSUM") as at_pool,
        ):
            # ---- persistent activations, split per s-tile chunk so the
            # framework's dependencies stay exact (a monolithic tile makes
            # every reader wait for the latest writer of ANY region).
            # Declared first so memsets can start at t=0 on idle engines.
            QT = [[pers.tile([P, SJ], QK_DT, name=f"QT{g}_{jj}")
                   for jj in range(NJ)] for g in range(2)]
            KT = [[[pers.tile([P, SJ], QK_DT, name=f"KT{g}{h2}_{jj}")
                    for jj in range(NJ)] for h2 in range(2)] for g in range(2)]
            V = [[pers.tile([P, NJ, 2, P], V_DT, name=f"V{g}_{jj}")
                  for jj in range(NJ)] for g in range(2)]
            attnG = [[pers.tile([P, SJ], V_DT, name=f"attnG{g}_{jj}")
                      for jj in range(NJ)] for g in range(2)]
            for g in range(2):
                for jj in range(NJ):
                    # pad rows hold ~1e-20: real data for the activity
                    # monitor, adds only ~1e-18 to each score. Split across
                    # DVE/GPSIMD so the head isn't serialized on one engine.
                    nc.vector.memset(KT[g][0][jj][E:, :], 1e-20)
                    nc.gpsimd.memset(KT[g][1][jj][0:E, :], 1e-20)
                    # pad cols (1.0) accumulate the softmax denominator into
                    # PSUM rows 64..127 during the attn matmul
                    (nc.gpsimd if jj % 2 else nc.vector).memset(
                        V[g][jj][:, :, :, E:], 1.0)

            # ---- weights + all x tiles up front (first-needed first) ----
            wq_sb = pers.tile([P, ND, 4 * E], QK_DT, name="wq_sb")
            nc.sync.dma_start(wq_sb[:], wq_d.ap())
            xq = [xt_pool.tile([P, ND, SJ], QK_DT, tag="xq", name=f"xq{j}")
                  for j in range(NJ)]
            xk = [xt_pool.tile([P, ND, SJ], QK_DT, tag="xk", name=f"xk{j}")
                  for j in range(NJ)]
            xv = [xt_pool.tile([P, ND, SJ], V_DT, tag="xv", name=f"xv{j}")
                  for j in range(NJ)]
            nc.sync.dma_start(xq[0][:], qT_d.ap()[:, 0])
            wk_sb = pers.tile([P, ND, 4 * E], QK_DT, name="wk_sb")
            nc.sync.dma_start(wk_sb[:], wk_d.ap())
            nc.sync.dma_start(xk[0][:], kT_d.ap()[:, 0])
            wv_sb = pers.tile([P, ND, 4 * E], V_DT, name="wv_sb")
            nc.sync.dma_start(wv_sb[:], wv_d.ap())
            nc.sync.dma_start(xv[0][:], vT_d.ap()[:, 0])
            wot_sb = pers.tile([P, 2, D], V_DT, name="wot_sb")
            nc.sync.dma_start(wot_sb[:], wot_d.ap())
            id_sb = pers.tile([P, P], QK_DT, name="id_sb")
            nc.sync.dma_start(id_sb[:], id_d.ap())
            npat_sb = pers.tile([P, P], QK_DT, name="npat_sb")
            nc.sync.dma_start(npat_sb[:], npat_d.ap())
            for j in range(1, NJ):
                nc.sync.dma_start(xq[j][:], qT_d.ap()[:, j])
                nc.sync.dma_start(xk[j][:], kT_d.ap()[:, j])
                nc.sync.dma_start(xv[j][:], vT_d.ap()[:, j])

            def epilogue(j, gs, atps):
                """Normalize attn by the softmax denominator for head pairs
                `gs` of tile j. All denominators sit on PSUM row 64; gather
                into one row-64 SBUF tile, one DMA to partitions 0..n-1, ONE
                reciprocal ([n,SJ] costs the same as [1,SJ] per-lane), then
                scatter rows 1.. back to partition 0 for the broadcasts
                (partition_broadcast input must be physical p0)."""
                n = 2 * len(gs)
                denb = epi_pool.tile([E + 1, 4, SJ], f32, tag="denb",
                                     name=f"denb{j}{gs[0]}")
                for i, (g, h2) in enumerate((g, h2) for g in gs for h2 in range(2)):
                    nc.vector.tensor_copy(denb[E:E + 1, i, :],
                                          atps[g][h2][E:E + 1, :])
                rec4 = epi_pool.tile([4, SJ], f32, tag="rec4",
                                     name=f"rec4{j}{gs[0]}")
                nc.sync.dma_start(rec4[0:n, :], denb[E:E + 1, 0:n, :])
                nc.vector.reciprocal(rec4[0:n, :], rec4[0:n, :])
                recs = [rec4]
                for i in range(1, n):
                    r = epi_pool.tile([1, SJ], f32, tag=f"recs{i}",
                                      name=f"recs{j}{gs[0]}{i}")
                    nc.sync.dma_start(r[:], rec4[i:i + 1, :])
                    recs.append(r)
                for i, (g, h2) in enumerate((g, h2) for g in gs for h2 in range(2)):
                    rec = recs[i][0:1, :]
                    recb = sm_pool.tile([E, SJ], f32, tag="recb",
                                        name=f"recb{g}{j}{h2}")
                    nc.gpsimd.partition_broadcast(recb[:], rec)
                    if h2 == 0:
                        nc.vector.tensor_tensor(
                            attnG[g][j][0:E, :], atps[g][h2][0:E, :], recb[:],
                            MULT)
                    else:
                        ah = sm_pool.tile([E, SJ], V_DT, tag="ah",
                                          name=f"ah{g}{j}")
                        nc.vector.tensor_tensor(
                            ah[:], atps[g][h2][0:E, :], recb[:], MULT)
                        nc.sync.dma_start(attnG[g][j][E:, :], ah[:])

            # ---- fused per-s-tile pipeline ----
            for j in range(NJ):
                js = slice(j * SJ, (j + 1) * SJ)
                for g in range(2):
                    pq = pj_pool.tile([P, SJ], f32, tag="pj", name=f"pq{j}{g}")
                    for c in range(ND):
                        nc.tensor.matmul(
                            pq[:], wq_sb[:, c, bass.ts(g, P)], xq[j][:, c, :],
                            start=(c == 0), stop=(c == ND - 1))
                    # on ACT (Copy shares the Exp table): keeps the
                    # scores(j)->QT dependency out of the DVE queue, which
                    # may still be draining the previous tile's epilogue
                    nc.scalar.copy(QT[g][j][:], pq[:])

                for g in range(2):
                    pk = pj_pool.tile([P, SJ], f32, tag="pj", name=f"pk{j}{g}")
                    for c in range(ND):
                        nc.tensor.matmul(
                            pk[:], wk_sb[:, c, bass.ts(g, P)], xk[j][:, c, :],
                            start=(c == 0), stop=(c == ND - 1))
                    nc.vector.tensor_copy(KT[g][0][j][0:E, :], pk[0:E, :])
                    nc.vector.tensor_copy(KT[g][1][j][E:, :], pk[E:, :])

                for u in range(SJ // P):
                    t = 4 * j + u
                    pv = pj_pool.tile([P, 2 * P], f32, tag="pj",
                                      name=f"pv{j}{u}")
                    for c in range(ND):
                        nc.tensor.matmul(
                            pv[:], xv[j][:, c, bass.ts(u, P)], wv_sb[:, c, :],
                            start=(c == 0), stop=(c == ND - 1))
                    for g in range(2):
                        for h2 in range(2):
                            lo = g * P + h2 * E
                            nc.vector.tensor_copy(
                                V[g][j][:, u, h2, 0:E], pv[:, lo:lo + E])

                # attention for both head pairs on this s-tile
                atps = {}
                for g in range(2):
                    if g == 1:
                        # normalize pair 0 while pair 1's attention still
                        # feeds the PE: the epilogue chain (copy->DMA->
                        # reciprocal->broadcast->mult) is ~8us of latency
                        epilogue(j, [0], atps)
                    nblk = 4 * j + 4
                    atp = [
                        at_pool.tile([P, SJ], f32, tag="at", name=f"at{g}{j}{h2}")
                        for h2 in range(2)
                    ]
                    atps[g] = atp
                    for cb in range(nblk):
                        col0 = max(0, cb - 4 * j) * P
                        scps = []
                        for h2 in range(2):
                            scp = sc_pool.tile(
                                [P, SJ], f32, tag="sc", name=f"sc{g}{j}{cb}{h2}")
                            diag = cb >= 4 * j
                            nc.tensor.matmul(
                                scp[:, col0:],
                                KT[g][h2][cb // 4][:, bass.ts(cb % 4, P)],
                                QT[g][j][:, col0:],
                                start=True, stop=not diag)
                            if diag:
                                # mask the upper triangle of the diagonal
                                # 128-strip by accumulating -400 (exp -> ~0)
                                # via identity x pattern on the PE: keeps the
                                # causal mask out of the DVE/ACT queues
                                nc.tensor.matmul(
                                    scp[:, col0:col0 + P], id_sb[:],
                                    npat_sb[:], start=False, stop=True)
                            scps.append(scp)
                        for h2 in range(2):
                            scp = scps[h2]
                            ex = ex_pool.tile(
                                [P, SJ], V_DT, tag="ex", name=f"ex{g}{j}{cb}{h2}")
                            nc.scalar.activation(
                                ex[:, col0:], scp[:, col0:], EXP, scale=1.0 / 32.0)
                            nc.tensor.matmul(
                                atp[h2][:, col0:],
                                V[g][cb // 4][:, cb % 4, h2, :],
                                ex[:, col0:],
                                start=(cb == 0), stop=(cb == nblk - 1))

                # ---- output projection for the PREVIOUS s-tile ----
                for u in range(SJ // P) if j > 0 else []:
                    si = 4 * (j - 1) + u
                    ot = ot_pool.tile([P, D], V_DT, tag="ot", name=f"ot{si}")
                    for no in range(2):
                        po = pj_pool.tile([P, SJ], f32, tag="pj",
                                          name=f"po{si}{no}")
                        for g in range(2):
                            nc.tensor.matmul(
                                po[:], attnG[g][si // 4][:, bass.ts(si % 4, P)],
                                wot_sb[:, g, bass.ts(no, SJ)],
                                start=(g == 0), stop=(g == 1))
                        nc.vector.tensor_copy(ot[:, bass.ts(no, SJ)], po[:])
                    nc.sync.dma_start(out_d.ap()[bass.ts(si, P), :], ot[:])

                # ---- softmax epilogue for pair 1 ----
                epilogue(j, [1], atps)

            # tail: output projection for the last s-tile
            for u in range(SJ // P):
                si = 4 * (NJ - 1) + u
                ot = ot_pool.tile([P, D], V_DT, tag="ot", name=f"ott{si}")
                for no in range(2):
                    po = pj_pool.tile([P, SJ], f32, tag="pj",
                                      name=f"pot{si}{no}")
                    for g in range(2):
                        nc.tensor.matmul(
                            po[:], attnG[g][si // 4][:, bass.ts(si % 4, P)],
                            wot_sb[:, g, bass.ts(no, SJ)],
                            start=(g == 0), stop=(g == 1))
                    nc.vector.tensor_copy(ot[:, bass.ts(no, SJ)], po[:])
                nc.sync.dma_start(out_d.ap()[bass.ts(si, P), :], ot[:])

    nc.compile()
    return nc


def _get_nc():
    if not _NC_CACHE:
        _NC_CACHE.append(_build())
    return _NC_CACHE[0]


def _tile_x(xT, np_dt):
    # [D, S] -> [P, NJ, ND, SJ]: x_t[p, j, o, s] = xT[o*P + p, j*SJ + s]
    t = xT.reshape(ND, P, NJ, SJ).transpose(1, 2, 0, 3)
    return np.ascontiguousarray(t).astype(np_dt)


def _in_maps(q, k, v, W_q, W_k, W_v, W_o):
    qk_np = _NP_OF[QK_DT]
    v_np = _NP_OF[V_DT]
    idm = np.eye(P, dtype=qk_np)
    npat = np.where(np.arange(P)[:, None] > np.arange(P)[None, :],
                    -400.0, 0.0).astype(qk_np)
    xT = {}
    for b in range(B):
        xT[b] = (
            _tile_x(np.ascontiguousarray(q[b].T), qk_np),
            _tile_x(np.ascontiguousarray(k[b].T), qk_np),
            _tile_x(np.ascontiguousarray(v[b].T), v_np),
        )

    def _tile_w(w):   # [D, 4E] -> [P, ND, 4E]
        return np.ascontiguousarray(w.reshape(ND, P, 4 * E).transpose(1, 0, 2))

    maps = []
    for core in range(NCORES):
        b, quad = divmod(core, 4)
        hs = slice(4 * quad, 4 * quad + 4)
        qT_b, kT_b, vT_b = xT[b]
        # [4, D, E] -> [D, 4, E] -> [D, 256], col l*64+e = W[4q+l, d, e]
        wq = W_q[hs].transpose(1, 0, 2).reshape(D, 4 * E)
        wk = W_k[hs].transpose(1, 0, 2).reshape(D, 4 * E)
        wv = W_v[hs].transpose(1, 0, 2).reshape(D, 4 * E)
        # W_o[out, in] -> W_o.T rows for this quad's 256 input dims
        wot = W_o[:, 4 * quad * E:4 * quad * E + 4 * E].T
        wot = wot.reshape(2, P, D).transpose(1, 0, 2)
        maps.append({
            "qT": qT_b,
            "kT": kT_b,
            "vT": vT_b,
            "wq": _tile_w(wq).astype(qk_np),
            "wk": _tile_w(wk).astype(qk_np),
            "wv": _tile_w(wv).astype(v_np),
            "wot": np.ascontiguousarray(wot).astype(v_np),
            "idm": idm,
            "npat": npat,
        })
    return maps


def kernel(q, k, v, W_q, W_k, W_v, W_o, _trace=False, _trace_kwargs=None):
    q = np.asarray(q, dtype=np.float32)
    k = np.asarray(k, dtype=np.float32)
    v = np.asarray(v, dtype=np.float32)
    W_q = np.asarray(W_q, dtype=np.float32)
    W_k = np.asarray(W_k, dtype=np.float32)
    W_v = np.asarray(W_v, dtype=np.float32)
    W_o = np.asarray(W_o, dtype=np.float32)

    nc = _get_nc()
    maps = _in_maps(q, k, v, W_q, W_k, W_v, W_o)
    kwargs = dict(_trace_kwargs or {})
    res = run_bass_kernel_spmd(
        nc, maps, core_ids=list(range(NCORES)), trace=_trace, **kwargs)
    out = np.zeros((B, S, D), dtype=np.float32)
    for core in range(NCORES):
        b = core // 4
        out[b] += res.results[core]["out"].astype(np.float32)
    if _trace:
        kernel.last_results = res
    return out


# revision 46
# speedup vs baseline: 1.0298x; 1.0298x over previous
"""Causal multi-head attention (nn_Attention_87840671138123) on 8 trn2 NeuronCores.

Problem (B=2, S=2048, D=1024, H=16 heads, E=64 head_dim), fp32:
    Q = einsum('bsd,hde->bhse', q, W_q)   (same for K, V)
    scores = Q @ K^T / sqrt(D), causal mask, softmax
    attn = probs @ V  -> [B, S, D] (head-major concat)
    out = attn @ W_o.T

Sharding: core = 4*b + quad. Each core handles batch b and a quad of 4 heads
(heads 4*quad .. 4*quad+3). It computes a partial output
    out_part = attn_quad @ W_o.T[quad rows, :]   [S, D]  (fp16)
and the host sums the 4 partials per batch (the "all-reduce" of the output
projection done host-side at gather time).

Device layout choices (per core):
 - Host pre-tiles x^T as [P, NJ, ND, SJ] so each j-tile DMA is one contiguous
   8KB segment per partition (big descriptors); all x tiles are pre-issued at
   kernel start so input DMA never gates compute.
 - QK path in bf16 (fp8 DoubleRow was measured at 1 cyc/col on this HW — no
   win — so bf16 at 1 cyc/col with full-width zero-padded KTH is optimal).
 - Projections produce QT in [head-pair x 64, S] layout and KTH per head
   zero-padded (1e-20) to 128 partitions so the scores matmul runs K=128.
 - V_aug blocks [64 V | 1.0 x 64]: the 1.0 columns accumulate the softmax
   denominator into PSUM rows 64..127 during the attn matmul.
 - Causal mask: the diagonal 128-strip accumulates a -400 upper-triangle
   pattern into the scores PSUM via an identity x pattern matmul on the PE
   (exp then gives ~e-12 relative weight) — no mask op on DVE/ACT/GPSIMD.
 - Softmax epilogue: the denominators of a pair land on PSUM row 64; both
   h2 rows are gathered into one row-64 SBUF tile, ONE DMA moves them to
   partitions 0..1 (partition_broadcast input must be PHYSICAL partition 0
   — GPSIMD DSPs only reach their own 16 partitions, verified on HW), ONE
   InstReciprocal covers both (per-lane cost, [2,SJ] == [1,SJ]), then
   broadcast + multiply. Pair 0's epilogue is emitted before pair 1's
   attention so its ~8us chain hides under PE work. Q-proj casts run on ACT
   (Copy shares the Exp activation table) so scores never wait for a DVE
   queue that may still be draining an epilogue.
 - Output projection one s-tile behind; fp16 partials DMA'd out.

HW findings baked into these choices (measured this session):
 - fp8 DoubleRow matmuls run at 1 cyc/col on this HW (not the cost model's
   0.5) -> no win over bf16; reverted.
 - walrus --enable-ldw-opt rejects the tile-framework's explicit Ldweights,
   and traces show Ldweights already fully overlap MATMULs without it.
 - InstReciprocal is ~6.4 ns/elem/lane (5x the cost-model estimate);
   reciprocal_approx_fast (custom DVE) returns garbage on this HW path.
 - partition_broadcast with input off physical p0 returns garbage (p64) or
   fails BIR verification (p1..p3).
 - DVE ops may read at most ONE PSUM operand; DMA cannot read PSUM.
"""

import ml_dtypes
import numpy as np

import concourse.bass as bass
import concourse.tile as tile
from concourse import bacc, mybir
from concourse.bass_utils import run_bass_kernel_spmd

B, S, D, H, E = 2, 2048, 1024, 16, 64
P = 128
NCORES = 8
SJ = 512            # s-tile width
NJ = S // SJ        # 4 s-tiles
ND = D // P         # 8 d-chunks
NT = S // P         # 16 t-chunks
f32 = mybir.dt.float32
bf16 = mybir.dt.bfloat16
fp16 = mybir.dt.float16
EXP = mybir.ActivationFunctionType.Exp
MULT = mybir.AluOpType.mult

QK_DT = bf16
V_DT = fp16

_NP_OF = {bf16: ml_dtypes.bfloat16, fp16: np.float16, f32: np.float32}

_NC_CACHE = []


def _patch_ldw_opt():
    """Enable walrus LDWEIGHTS optimization (fast weight load)."""
    from concourse import bass_utils as _bu
    if getattr(_bu, "_ldw_patched", False):
        return
    _orig_run = _bu.run_command

    def _run(argv, **kw):
        argv = ["--enable-ldw-opt=true" if a == "--enable-ldw-opt=false" else a
                for a in argv]
        return _orig_run(argv, **kw)

    _bu.run_command = _run
    _bu._ldw_patched = True


def _build():
    # NOTE: walrus --enable-ldw-opt rejects this kernel's explicit
    # InstLdweights stream, and traces show the tile-framework Ldweights
    # already overlap fully with MATMULs without it — so no patch here.
    nc = bacc.Bacc("TRN2", target_bir_lowering=False, debug=False)

    qT_d = nc.dram_tensor("qT", [P, NJ, ND, SJ], QK_DT, kind="ExternalInput")
    kT_d = nc.dram_tensor("kT", [P, NJ, ND, SJ], QK_DT, kind="ExternalInput")
    vT_d = nc.dram_tensor("vT", [P, NJ, ND, SJ], V_DT, kind="ExternalInput")
    wq_d = nc.dram_tensor("wq", [P, ND, 4 * E], QK_DT, kind="ExternalInput")
    wk_d = nc.dram_tensor("wk", [P, ND, 4 * E], QK_DT, kind="ExternalInput")
    wv_d = nc.dram_tensor("wv", [P, ND, 4 * E], V_DT, kind="ExternalInput")
    wot_d = nc.dram_tensor("wot", [P, 2, D], V_DT, kind="ExternalInput")
    id_d = nc.dram_tensor("idm", [P, P], QK_DT, kind="ExternalInput")
    npat_d = nc.dram_tensor("npat", [P, P], QK_DT, kind="ExternalInput")
    out_d = nc.dram_tensor("out", [S, D], fp16, kind="ExternalOutput")

    with tile.TileContext(nc) as tc:
        with (
            tc.tile_pool(name="pers", bufs=1) as pers,
            tc.tile_pool(name="xt", bufs=NJ) as xt_pool,
            tc.tile_pool(name="ex", bufs=4) as ex_pool,
            tc.tile_pool(name="sm", bufs=2) as sm_pool,
            tc.tile_pool(name="epi", bufs=1) as epi_pool,
            tc.tile_pool(name="ot", bufs=2) as ot_pool,
            tc.tile_pool(name="pj", bufs=2, space="PSUM") as pj_pool,
            tc.tile_pool(name="sc", bufs=2, space="PSUM") as sc_pool,
            tc.tile_pool(name="at", bufs=4, space="PSUM") as at_pool,
        ):
            # ---- persistent activations, split per s-tile chunk so the
            # framework's dependencies stay exact (a monolithic tile makes
            # every reader wait for the latest writer of ANY region).
            # Declared first so memsets can start at t=0 on idle engines.
            QT = [[pers.tile([P, SJ], QK_DT, name=f"QT{g}_{jj}")
                   for jj in range(NJ)] for g in range(2)]
            KT = [[[pers.tile([P, SJ], QK_DT, name=f"KT{g}{h2}_{jj}")
                    for jj in range(NJ)] for h2 in range(2)] for g in range(2)]
            V = [[pers.tile([P, NJ, 2, P], V_DT, name=f"V{g}_{jj}")
                  for jj in range(NJ)] for g in range(2)]
            attnG = [[pers.tile([P, SJ], V_DT, name=f"attnG{g}_{jj}")
                      for jj in range(NJ)] for g in range(2)]
            for g in range(2):
                for jj in range(NJ):
                    # pad rows hold ~1e-20: real data for the activity
                    # monitor, adds only ~1e-18 to each score. Split across
                    # DVE/GPSIMD so the head isn't serialized on one engine.
                    nc.vector.memset(KT[g][0][jj][E:, :], 1e-20)
                    nc.gpsimd.memset(KT[g][1][jj][0:E, :], 1e-20)
                    # pad cols (1.0) accumulate the softmax denominator into
                    # PSUM rows 64..127 during the attn matmul
                    (nc.gpsimd if jj % 2 else nc.vector).memset(
                        V[g][jj][:, :, :, E:], 1.0)

            # ---- weights + all x tiles up front (first-needed first) ----
            wq_sb = pers.tile([P, ND, 4 * E], QK_DT, name="wq_sb")
            nc.sync.dma_start(wq_sb[:], wq_d.ap())
            xq = [xt_pool.tile([P, ND, SJ], QK_DT, tag="xq", name=f"xq{j}")
                  for j in range(NJ)]
            xk = [xt_pool.tile([P, ND, SJ], QK_DT, tag="xk", name=f"xk{j}")
                  for j in range(NJ)]
            xv = [xt_pool.tile([P, ND, SJ], V_DT, tag="xv", name=f"xv{j}")
                  for j in range(NJ)]
            nc.sync.dma_start(xq[0][:], qT_d.ap()[:, 0])
            wk_sb = pers.tile([P, ND, 4 * E], QK_DT, name="wk_sb")
            nc.sync.dma_start(wk_sb[:], wk_d.ap())
            nc.sync.dma_start(xk[0][:], kT_d.ap()[:, 0])
            wv_sb = pers.tile([P, ND, 4 * E], V_DT, name="wv_sb")
            nc.sync.dma_start(wv_sb[:], wv_d.ap())
            nc.sync.dma_start(xv[0][:], vT_d.ap()[:, 0])
            wot_sb = pers.tile([P, 2, D], V_DT, name="wot_sb")
            nc.sync.dma_start(wot_sb[:], wot_d.ap())
            id_sb = pers.tile([P, P], QK_DT, name="id_sb")
            nc.sync.dma_start(id_sb[:], id_d.ap())
            npat_sb = pers.tile([P, P], QK_DT, name="npat_sb")
            nc.sync.dma_start(npat_sb[:], npat_d.ap())
            for j in range(1, NJ):
                nc.sync.dma_start(xq[j][:], qT_d.ap()[:, j])
                nc.sync.dma_start(xk[j][:], kT_d.ap()[:, j])
                nc.sync.dma_start(xv[j][:], vT_d.ap()[:, j])

            def epilogue(j, gs, atps):
                """Normalize attn by the softmax denominator for head pairs
                `gs` of tile j. All denominators sit on PSUM row 64; gather
                into one row-64 SBUF tile, one DMA to partitions 0..n-1, ONE
                reciprocal ([n,SJ] costs the same as [1,SJ] per-lane), then
                scatter rows 1.. back to partition 0 for the broadcasts
                (partition_broadcast input must be physical p0)."""
                n = 2 * len(gs)
                denb = epi_pool.tile([E + 1, 4, SJ], f32, tag="denb",
                                     name=f"denb{j}{gs[0]}")
                for i, (g, h2) in enumerate((g, h2) for g in gs for h2 in range(2)):
                    nc.vector.tensor_copy(denb[E:E + 1, i, :],
                                          atps[g][h2][E:E + 1, :])
                rec4 = epi_pool.tile([4, SJ], f32, tag="rec4",
                                     name=f"rec4{j}{gs[0]}")
                nc.sync.dma_start(rec4[0:n, :], denb[E:E + 1, 0:n, :])
                nc.vector.reciprocal(rec4[0:n, :], rec4[0:n, :])
                recs = [rec4]
                for i in range(1, n):
                    r = epi_pool.tile([1, SJ], f32, tag=f"recs{i}",
                                      name=f"recs{j}{gs[0]}{i}")
                    nc.sync.dma_start(r[:], rec4[i:i + 1, :])
                    recs.append(r)
                for i, (g, h2) in enumerate((g, h2) for g in gs for h2 in range(2)):
                    rec = recs[i][0:1, :]
                    recb = sm_pool.tile([E, SJ], f32, tag="recb",
                                        name=f"recb{g}{j}{h2}")
                    nc.gpsimd.partition_broadcast(recb[:], rec)
                    if h2 == 0:
                        nc.vector.tensor_tensor(
                            attnG[g][j][0:E, :], atps[g][h2][0:E, :], recb[:],
                            MULT)
                    else:
                        ah = sm_pool.tile([E, SJ], V_DT, tag="ah",
                                          name=f"ah{g}{j}")
                        nc.vector.tensor_tensor(
                            ah[:], atps[g][h2][0:E, :], recb[:], MULT)
                        nc.sync.dma_start(attnG[g][j][E:, :], ah[:])

            # ---- fused per-s-tile pipeline ----
            for j in range(NJ):
                js = slice(j * SJ, (j + 1) * SJ)
                for g in range(2):
                    pq = pj_pool.tile([P, SJ], f32, tag="pj", name=f"pq{j}{g}")
                    for c in range(ND):
                        nc.tensor.matmul(
                            pq[:], wq_sb[:, c, bass.ts(g, P)], xq[j][:, c, :],
                            start=(c == 0), stop=(c == ND - 1))
                    # on ACT (Copy shares the Exp table): keeps the
                    # scores(j)->QT dependency out of the DVE queue, which
                    # may still be draining the previous tile's epilogue
                    nc.scalar.copy(QT[g][j][:], pq[:])

                for g in range(2):
                    pk = pj_pool.tile([P, SJ], f32, tag="pj", name=f"pk{j}{g}")
                    for c in range(ND):
                        nc.tensor.matmul(
                            pk[:], wk_sb[:, c, bass.ts(g, P)], xk[j][:, c, :],
                            start=(c == 0), stop=(c == ND - 1))
                    nc.vector.tensor_copy(KT[g][0][j][0:E, :], pk[0:E, :])
                    nc.vector.tensor_copy(KT[g][1][j][E:, :], pk[E:, :])

                for u in range(SJ // P):
                    t = 4 * j + u
                    pv = pj_pool.tile([P, 2 * P], f32, tag="pj",
                                      name=f"pv{j}{u}")
                    for c in range(ND):
                        nc.tensor.matmul(
                            pv[:], xv[j][:, c, bass.ts(u, P)], wv_sb[:, c, :],
                            start=(c == 0), stop=(c == ND - 1))
                    for g in range(2):
                        for h2 in range(2):
                            lo = g * P + h2 * E
                            nc.vector.tensor_copy(
                                V[g][j][:, u, h2, 0:E], pv[:, lo:lo + E])

                # attention for both head pairs on this s-tile
                atps = {}
                for g in range(2):
                    if g == 1:
                        # normalize pair 0 while pair 1's attention still
                        # feeds the PE: the epilogue chain (copy->DMA->
                        # reciprocal->broadcast->mult) is ~8us of latency
                        epilogue(j, [0], atps)
                    nblk = 4 * j + 4
                    atp = [
                        at_pool.tile([P, SJ], f32, tag="at", name=f"at{g}{j}{h2}")
                        for h2 in range(2)
                    ]
                    atps[g] = atp
                    for cb in range(nblk):
                        col0 = max(0, cb - 4 * j) * P
                        scps = []
                        for h2 in range(2):
                            scp = sc_pool.tile(
                                [P, SJ], f32, tag="sc", name=f"sc{g}{j}{cb}{h2}")
                            diag = cb >= 4 * j
                            nc.tensor.matmul(
                                scp[:, col0:],
                                KT[g][h2][cb // 4][:, bass.ts(cb % 4, P)],
                                QT[g][j][:, col0:],
                                start=True, stop=not diag)
                            if diag:
                                # mask the upper triangle of the diagonal
                                # 128-strip by accumulating -400 (exp -> ~0)
                                # via identity x pattern on the PE: keeps the
                                # causal mask out of the DVE/ACT queues
                                nc.tensor.matmul(
                                    scp[:, col0:col0 + P], id_sb[:],
                                    npat_sb[:], start=False, stop=True)
                            scps.append(scp)
                        for h2 in range(2):
                            scp = scps[h2]
                            ex = ex_pool.tile(
                                [P, SJ], V_DT, tag="ex", name=f"ex{g}{j}{cb}{h2}")
                            nc.scalar.activation(
                                ex[:, col0:], scp[:, col0:], EXP, scale=1.0 / 32.0)
                            nc.tensor.matmul(
                                atp[h2][:, col0:],
                                V[g][cb // 4][:, cb % 4, h2, :],
                                ex[:, col0:],
                                start=(cb == 0), stop=(cb == nblk - 1))

                # ---- output projection for the PREVIOUS s-tile ----
                for u in range(SJ // P) if j > 0 else []:
                    si = 4 * (j - 1) + u
                    ot = ot_pool.tile([P, D], V_DT, tag="ot", name=f"ot{si}")
                    for no in range(2):
                        po = pj_pool.tile([P, SJ], f32, tag="pj",
                                          name=f"po{si}{no}")
                        for g in range(2):
                            nc.tensor.matmul(
                                po[:], attnG[g][si // 4][:, bass.ts(si % 4, P)],
                                wot_sb[:, g, bass.ts(no, SJ)],
                                start=(g == 0), stop=(g == 1))
                        nc.vector.tensor_copy(ot[:, bass.ts(no, SJ)], po[:])
                    nc.sync.dma_start(out_d.ap()[bass.ts(si, P), :], ot[:])

                # ---- softmax epilogue for pair 1 ----
                epilogue(j, [1], atps)

            # tail: output projection for the last s-tile
            for u in range(SJ // P):
                si = 4 * (NJ - 1) + u
                ot = ot_pool.tile([P, D], V_DT, tag="ot", name=f"ott{si}")
                for no in range(2):
                    po = pj_pool.tile([P, SJ], f32, tag="pj",
                                      name=f"pot{si}{no}")
                    for g in range(2):
                        nc.tensor.matmul(
                            po[:], attnG[g][si // 4][:, bass.ts(si % 4, P)],
                            wot_sb[:, g, bass.ts(no, SJ)],
                            start=(g == 0), stop=(g == 1))
                    nc.vector.tensor_copy(ot[:, bass.ts(no, SJ)], po[:])
                nc.sync.dma_start(out_d.ap()[bass.ts(si, P), :], ot[:])

    nc.compile()
    return nc


def _get_nc():
    if not _NC_CACHE:
        _NC_CACHE.append(_build())
    return _NC_CACHE[0]


def _tile_x(xT, np_dt):
    # [D, S] -> [P, NJ, ND, SJ]: x_t[p, j, o, s] = xT[o*P + p, j*SJ + s]
    t = xT.reshape(ND, P, NJ, SJ).transpose(1, 2, 0, 3)
    return np.ascontiguousarray(t).astype(np_dt)


def _in_maps(q, k, v, W_q, W_k, W_v, W_o):
    qk_np = _NP_OF[QK_DT]
    v_np = _NP_OF[V_DT]
    idm = np.eye(P, dtype=qk_np)
    npat = np.where(np.arange(P)[:, None] > np.arange(P)[None, :],
                    -400.0, 0.0).astype(qk_np)
    xT = {}
    for b in range(B):
        xT[b] = (
            _tile_x(np.ascontiguousarray(q[b].T), qk_np),
            _tile_x(np.ascontiguousarray(k[b].T), qk_np),
            _tile_x(np.ascontiguousarray(v[b].T), v_np),
        )

    def _tile_w(w):   # [D, 4E] -> [P, ND, 4E]
        return np.ascontiguousarray(w.reshape(ND, P, 4 * E).transpose(1, 0, 2))

    maps = []
    for core in range(NCORES):
        b, quad = divmod(core, 4)
        hs = slice(4 * quad, 4 * quad + 4)
        qT_b, kT_b, vT_b = xT[b]
        # [4, D, E] -> [D, 4, E] -> [D, 256], col l*64+e = W[4q+l, d, e]
        wq = W_q[hs].transpose(1, 0, 2).reshape(D, 4 * E)
        wk = W_k[hs].transpose(1, 0, 2).reshape(D, 4 * E)
        wv = W_v[hs].transpose(1, 0, 2).reshape(D, 4 * E)
        # W_o[out, in] -> W_o.T rows for this quad's 256 input dims
        wot = W_o[:, 4 * quad * E:4 * quad * E + 4 * E].T
        wot = wot.reshape(2, P, D).transpose(1, 0, 2)
        maps.append({
            "qT": qT_b,
            "kT": kT_b,
            "vT": vT_b,
            "wq": _tile_w(wq).astype(qk_np),
            "wk": _tile_w(wk).astype(qk_np),
            "wv": _tile_w(wv).astype(v_np),
            "wot": np.ascontiguousarray(wot).astype(v_np),
            "idm": idm,
            "npat": npat,
        })
    return maps


def kernel(q, k, v, W_q, W_k, W_v, W_o, _trace=False, _trace_kwargs=None):
    q = np.asarray(q, dtype=np.float32)
    k = np.asarray(k, dtype=np.float32)
    v = np.asarray(v, dtype=np.float32)
    W_q = np.asarray(W_q, dtype=np.float32)
    W_k = np.asarray(W_k, dtype=np.float32)
    W_v = np.asarray(W_v, dtype=np.float32)
    W_o = np.asarray(W_o, dtype=np.float32)

    nc = _get_nc()
    maps = _in_maps(q, k, v, W_q, W_k, W_v, W_o)
    kwargs = dict(_trace_kwargs or {})
    res = run_bass_kernel_spmd(
        nc, maps, core_ids=list(range(NCORES)), trace=_trace, **kwargs)
    out = np.zeros((B, S, D), dtype=np.float32)
    for core in range(NCORES):
        b = core // 4
        out[b] += res.results[core]["out"].astype(np.float32)
    if _trace:
        kernel.last_results = res
    return out
